# revision 48
# baseline (speedup 1.0000x reference)
"""Distributed manual-attention kernel for Trainium2 (8 NeuronCores).

Problem: q,k,v (128, 8192) f32; out = softmax(q^T k, axis=kv) @ v^T -> (8192, 128).

Strategy: shard seqlen_q across the 8 cores (1024 q columns each); k/v are
replicated.  Each core runs an independent flash-attention-style kernel:

  for each q-chunk (512 q):
    for each kv batch b (up to 3 tiles of 128 kv):
      S^T[b]   = k_tile^T @ q_chunk          (PE, bf16, out (kv, q) f32 PSUM)
      E[b]     = exp(S^T[b] - 60)            (ACT, bf16 out, bias rides free affine)
      outT    += vT_tile^T @ E[b]            (PE, bf16, accum (d, q) f32 PSUM)
      chain[i] += E[b]                       (DVE, bf16 2x mode)
    den[q]    = ones-matmuls over chains + late tiles   (PE, f32 accum)
    out       = transpose(outT) * 1/den      (PE transpose + split-engine scale)

v is fed to the device PRE-TRANSPOSED on the host (vt[p, 128t+c] = v[c, 128t+p],
the exact SBUF layout mm2's stationary operand wants): zero device transposes
of v.  Inputs arrive as bf16 via gpsimd-initiated CASTING DMAs (f32 HBM ->
bf16 SBUF in flight); q0 and two early k pieces ride the HWDGE path (sync
queue, f32 + DVE cast) in parallel with the gpsimd stream, halving the
staircase's delivery latency.

ACT (exp) is the pacing engine: 65536 exp elems per partition per core at
~1 elem/cycle.  Everything else hides underneath it:
  - chunk 0 staircases in (1,1,2,2,3...) with k DMA pieces cut to match, so
    the exp stream starts early and stays dense while the software DMA
    engine's ~3us transfer latency catches up; mm2 batches run at lag-2
    early on so a late vt piece never head-of-line blocks an mm1.
  - the denominator engine is chosen per chunk by which engine has slack
    where that chunk's epilogue lands.  Chunk 0's epilogue overlaps chunk
    1's steady state, where the DVE idles ~0.8us/window but the PE only
    ~0.25us: its chain folds on the DVE (adds -> PE transpose -> reduce ->
    reciprocal).  Chunk 1's epilogue IS the tail, where latency rules: its
    chain collapses via accumulating ones-matmuls (ones^T @ acc / E) into
    a (1,512) PSUM row closed before the last exp, extracted to
    q-partitions by four K=1 matmuls; the final 1-tile batch enters the
    extract tile directly via transposed ones-matmuls (E_slice^T @ ones),
    one DVE reduce folds the halves -- after the last exp only ~1us of
    work gates the reciprocal.
  - at the chunk boundary both chunks' mm2 backlogs are deferred behind
    chunk 1's early mm1s; chunk 1 opens with a 1-tile batch.
  - the tail splits across engines: den-copy + cast + recip + 2 scales on
    DVE, 2 scales on Scalar, den/extract/transpose matmuls on PE, with
    each output DMA issued right after its two scales.

exp is computed as exp(qk - 60): softmax is shift-invariant and row maxima
of qk reach ~117 > ln(f32_max)=88.7, so unshifted exp overflows f32 on ~2%
of rows.  With the shift, exp <= e^57: safe in f32 and bf16.
"""

import numpy as np

D = 128          # head dim
SQ = 8192        # total seqlen_q
SKV = 8192       # seqlen_kv
NCORES = 8
SQS = SQ // NCORES   # 1024 q per core
QC = 512             # q chunk (matmul moving free dim)
NQC = SQS // QC      # 2 chunks
KVT = 128            # kv tile (PE contraction / partition dim)
NKV = SKV // KVT     # 64 kv tiles
N_WARMUP = 4         # PE warm-up matmuls (HAM ramp)

# kv-tile batch sizes per chunk (sum = NKV).
BATCHES_C0 = [1, 1, 2, 2] + [3] * 19 + [1]
BATCHES_C1 = [1] + [3] * 20 + [2, 1]

# k DMA pieces (col ranges) cut to the chunk-0 consumption staircase.
K_PIECES = [(0, 128), (128, 256), (256, 512), (512, 768), (768, 1152),
            (1152, 1536)] + [
    (1536 + 512 * i, 1536 + 512 * (i + 1)) for i in range(13)
]
VT_PIECES = [(512 * i, 512 * (i + 1)) for i in range(16)]

LAST_RESULTS = None  # BassKernelResults of the most recent run (for test.py)


def _build_nc():
    import concourse.tile as tile
    from concourse import bacc, mybir
    from concourse.masks import make_identity

    f32 = mybir.dt.float32
    bf16 = mybir.dt.bfloat16

    nc = bacc.Bacc(None, target_bir_lowering=False)
    q_ext = nc.declare_dram_parameter("q", [D, SQS], f32, isOutput=False)
    k_ext = nc.declare_dram_parameter("k", [D, SKV], f32, isOutput=False)
    vt_ext = nc.declare_dram_parameter("vt", [D, SKV], f32, isOutput=False)
    out_ext = nc.declare_dram_parameter("out", [SQS, D], f32, isOutput=True)

    def mk_batches(sizes):
        out, t = [], 0
        for s in sizes:
            out.append(list(range(t, t + s)))
            t += s
        assert t == NKV
        return out

    batches_by_chunk = [mk_batches(BATCHES_C0), mk_batches(BATCHES_C1)]

    with tile.TileContext(nc) as tc:
        with (
            tc.tile_pool(name="const", bufs=1) as constp,
            tc.tile_pool(name="inputs", bufs=1) as inputs,
            tc.tile_pool(name="work", bufs=7) as workp,
            tc.tile_pool(name="accp", bufs=2) as accp,
            tc.tile_pool(name="epi", bufs=2) as epip,
            tc.tile_pool(name="qk_ps", bufs=2, space="PSUM") as qkps,
            tc.tile_pool(name="out_ps", bufs=1, space="PSUM") as outps,
            tc.tile_pool(name="misc_ps", bufs=1, space="PSUM") as miscps,
        ):
            # ---- lead-in ----------------------------------------------
            k0_tile = inputs.tile([D, K_PIECES[0][1]], bf16, name="k0",
                                  tag="k0")
            nc.gpsimd.dma_start(out=k0_tile, in_=k_ext[:, 0:K_PIECES[0][1]])
            scratch = constp.tile([128, 512], bf16, name="scratch")
            nc.gpsimd.memset(scratch, 0.0)
            bias_m60 = constp.tile([128, 1], f32, name="bias_m60")
            nc.gpsimd.memset(bias_m60, -60.0)
            dummy = constp.tile([128, 1], f32, name="dummy")
            nc.scalar.activation(dummy, bias_m60,
                                 func=mybir.ActivationFunctionType.Exp)
            warm_ps = outps.tile([128, 512], f32, tag="outT", name="warm_ps")
            for _ in range(N_WARMUP):
                nc.tensor.matmul(
                    warm_ps, lhsT=scratch[:, 0:128], rhs=scratch,
                    start=True, stop=True,
                )

            q_tiles = [inputs.tile([D, QC], bf16, name=f"q{c}", tag=f"q{c}")
                       for c in range(NQC)]
            k_pieces = [k0_tile] + [
                inputs.tile([D, hi - lo], bf16, name=f"k{i}", tag=f"k{i}")
                for i, (lo, hi) in enumerate(K_PIECES) if i > 0
            ]
            vt_pieces = [
                inputs.tile([D, hi - lo], bf16, name=f"vt{i}", tag=f"vt{i}")
                for i, (lo, hi) in enumerate(VT_PIECES)
            ]

            # q0 and k1/k3 ride the HWDGE path (sync queue, f32 + DVE cast)
            # IN PARALLEL with k0/k2/k4 on the gpsimd casting queue: two DMA
            # paths halve the staircase's serialized delivery latency.
            q0_f32 = inputs.tile([D, QC], f32, name="q0f", tag="q0f")
            nc.sync.dma_start(out=q0_f32, in_=q_ext[:, 0:QC])
            nc.vector.tensor_copy(q_tiles[0], q0_f32)

            def dma_k(i):
                lo, hi = K_PIECES[i]
                nc.gpsimd.dma_start(out=k_pieces[i], in_=k_ext[:, lo:hi])

            def dma_k_sync(i):
                lo, hi = K_PIECES[i]
                kf = inputs.tile([D, hi - lo], f32, name=f"kf{i}",
                                 tag=f"kf{i}")
                nc.sync.dma_start(out=kf, in_=k_ext[:, lo:hi])
                nc.vector.tensor_copy(k_pieces[i], kf)

            def dma_vt(i):
                lo, hi = VT_PIECES[i]
                nc.gpsimd.dma_start(out=vt_pieces[i], in_=vt_ext[:, lo:hi])

            dma_k_sync(1)
            dma_k(2)
            dma_k_sync(3)
            dma_vt(0)
            dma_k(4)
            ki, vi = 5, 1
            for step in range(16):
                if vi < 16:
                    dma_vt(vi)
                    vi += 1
                if ki < len(K_PIECES):
                    dma_k(ki)
                    ki += 1
                if step == 4:
                    nc.gpsimd.dma_start(out=q_tiles[1],
                                        in_=q_ext[:, QC:2 * QC])
            assert ki == len(K_PIECES) and vi == 16

            # constants for the epilogue (needed only mid-kernel)
            ident_bf = constp.tile([128, 128], bf16, name="ident_bf")
            make_identity(nc, ident_bf)
            ones_col = constp.tile([128, 1], bf16, name="ones_col")
            nc.gpsimd.memset(ones_col, 1.0)
            ones_1 = constp.tile([1, 1], bf16, name="ones_1")
            nc.gpsimd.memset(ones_1, 1.0)

            # ---- lhsT lookups ------------------------------------------
            k_start = [lo for lo, _ in K_PIECES]

            def mm1_lhsT(t):
                col = t * KVT
                for i in range(len(K_PIECES) - 1, -1, -1):
                    if k_start[i] <= col:
                        off = col - k_start[i]
                        return k_pieces[i][:, off:off + KVT]
                raise AssertionError

            def mm2_lhsT(t):
                return vt_pieces[t // 4][:, (t % 4) * KVT:(t % 4) * KVT + KVT]

            # ---- per-chunk state ---------------------------------------
            class Chunk:
                pass

            def start_chunk(c):
                st = Chunk()
                st.c = c
                st.batches = batches_by_chunk[c]
                st.nb = len(st.batches)
                # last 2 batches bypass the chains; the final one bypasses
                # even the den row (transposed matmuls in the tail)
                st.direct = {st.nb - 2, st.nb - 1}
                chained = [b for b in range(st.nb) if b not in st.direct]
                st.chain_of = {b: 0 for b in chained}
                st.chain_prev = [None]
                st.chain_live = [False]
                st.chain_width = [0]
                st.chain_last = max(chained)
                st.q_rhs = q_tiles[c]
                st.outT_ps = outps.tile([128, QC], f32, tag="outT",
                                        name=f"outT{c}")
                st.accs = [
                    accp.tile([128, 3 * QC], bf16, tag="acc0",
                              name=f"acc{c}_0")
                ]
                st.stashed = {}
                st.mm2_pending = []
                # den row matmul count: the acc's slices + the
                # second-to-last batch's tiles (the final batch goes
                # straight into the extract tile via transposed matmuls)
                st.den_total = 3 + len(st.batches[st.nb - 2])
                st.den_emitted = 0
                st.den_ps = None
                return st

            def flush_mm2(st, count=None):
                n = len(st.mm2_pending) if count is None else count
                for batch, exp3 in st.mm2_pending[:n]:
                    for j, t in enumerate(batch):
                        nc.tensor.matmul(
                            st.outT_ps,
                            lhsT=mm2_lhsT(t),
                            rhs=exp3[:, j * QC:(j + 1) * QC],
                            start=(t == 0),
                            stop=(t == NKV - 1),
                        )
                del st.mm2_pending[:n]

            def den_mm(st, rhs512):
                # accumulate ones^T @ rhs into this chunk's den row (PE)
                if st.den_ps is None:
                    st.den_ps = miscps.tile([1, QC], f32, tag="misc",
                                            name=f"den{st.c}")
                nc.tensor.matmul(
                    st.den_ps, lhsT=ones_col, rhs=rhs512,
                    start=(st.den_emitted == 0),
                    stop=(st.den_emitted == st.den_total - 1),
                )
                st.den_emitted += 1

            def den_mm_acc(st, j):
                for sl in range(3):
                    den_mm(st, st.accs[j][:, sl * QC:(sl + 1) * QC])

            def den_mm_exp(st, bi):
                e = st.stashed[bi]
                for sl in range(len(st.batches[bi])):
                    den_mm(st, e[:, sl * QC:(sl + 1) * QC])

            def emit_chain(st, bi, exp3, w):
                # A narrower tile adds into the accumulator's low columns
                # only (all columns are summed by the den matmuls anyway);
                # a wider tile extends the accumulator's live width.
                ch = st.chain_of[bi]
                acc = st.accs[ch]
                if st.chain_prev[ch] is None and not st.chain_live[ch]:
                    st.chain_prev[ch] = (exp3, w)
                    return
                if not st.chain_live[ch]:
                    pexp, pw = st.chain_prev[ch]
                    lo = min(pw, w)
                    nc.vector.tensor_add(acc[:, :lo], pexp[:, :lo],
                                         exp3[:, :lo])
                    if w > pw:
                        nc.vector.tensor_copy(acc[:, pw:w], exp3[:, pw:w])
                    elif pw > w:
                        nc.vector.tensor_copy(acc[:, w:pw], pexp[:, w:pw])
                    st.chain_prev[ch] = None
                    st.chain_live[ch] = True
                    st.chain_width[ch] = max(pw, w)
                    return
                cw = st.chain_width[ch]
                lo = min(cw, w)
                nc.vector.tensor_add(acc[:, :lo], acc[:, :lo], exp3[:, :lo])
                if w > cw:
                    nc.vector.tensor_copy(acc[:, cw:w], exp3[:, cw:w])
                    st.chain_width[ch] = w

            def emit_batch(st, bi, flush=True, flush_count=None):
                c = st.c
                batch = st.batches[bi]
                w = len(batch) * QC
                qk_ps = qkps.tile([128, 3 * QC], f32, tag="qk",
                                  name=f"qk{c}_{bi}")
                for j, t in enumerate(batch):
                    nc.tensor.matmul(
                        qk_ps[:, j * QC:(j + 1) * QC],
                        lhsT=mm1_lhsT(t),
                        rhs=st.q_rhs,
                        start=True,
                        stop=True,
                    )
                exp3 = workp.tile([128, 3 * QC], bf16, tag="exp3",
                                  name=f"exp{c}_{bi}")
                nc.scalar.activation(
                    exp3[:, :w], qk_ps[:, :w],
                    func=mybir.ActivationFunctionType.Exp,
                    bias=bias_m60,
                )
                if flush:
                    flush_mm2(st, flush_count)
                if bi in st.direct:
                    st.stashed[bi] = exp3
                else:
                    emit_chain(st, bi, exp3, w)
                # dribbled den-row matmuls, each placed one batch after its
                # source engine finished so the PE queue never waits on them
                if bi == st.chain_last + 2 and st.c == NQC - 1:
                    den_mm_acc(st, 0)
                if bi == st.nb - 1 and st.c == NQC - 1:
                    # the pre-final direct batch's row matmuls: its exp is
                    # done, so the row closes during this (last) exp
                    den_mm_exp(st, st.nb - 2)
                if bi == st.nb - 1 and st.den_emitted == st.den_total:
                    # den row complete: pull it to SBUF while the last exp
                    # still runs (its own tile enters via epi_den4 instead)
                    epi_den_copy(st)
                st.mm2_pending.append((batch, exp3))
                if bi == st.nb - 1 and flush:
                    flush_mm2(st)
                    if c < NQC - 1:
                        epi_cast(st)

            # ---- epilogue stages ---------------------------------------
            def epi_fold0(st, part):
                # chunk-0 denominator on the DVE (its epilogue overlaps
                # chunk 1, where the DVE has slack and the PE does not):
                # fold the chain + direct tiles to a 512-wide acc_sum
                acc = st.accs[0]
                if part == 0:
                    st.acc_sum = epip.tile([128, QC], bf16, tag="acc_sum",
                                           name=f"accs{st.c}")
                    nc.vector.tensor_add(st.acc_sum, acc[:, 0:QC],
                                         acc[:, QC:2 * QC])
                    nc.vector.tensor_add(st.acc_sum, st.acc_sum,
                                         acc[:, 2 * QC:3 * QC])
                else:
                    e2 = st.stashed[st.nb - 2]
                    for sl in range(len(st.batches[st.nb - 2])):
                        nc.vector.tensor_add(st.acc_sum, st.acc_sum,
                                             e2[:, sl * QC:(sl + 1) * QC])
                    nc.vector.tensor_add(st.acc_sum, st.acc_sum,
                                         st.stashed[st.nb - 1][:, 0:QC])

            def epi_denom0(st):
                accT_ps = miscps.tile([128, QC], bf16, tag="misc",
                                      name=f"accT{st.c}")
                for s in range(4):
                    nc.tensor.transpose(
                        accT_ps[:, s * 128:(s + 1) * 128],
                        st.acc_sum[:, s * 128:(s + 1) * 128],
                        ident_bf,
                    )
                denom4 = epip.tile([128, 4], f32, tag="denom4",
                                   name=f"den4s{st.c}")
                nc.vector.tensor_reduce(
                    denom4,
                    accT_ps.rearrange("p (s j) -> p s j", s=4),
                    axis=mybir.AxisListType.X,
                    op=mybir.AluOpType.add,
                )
                st.recip4 = epip.tile([128, 4], f32, tag="recip4",
                                      name=f"rec{st.c}")
                nc.vector.reciprocal(st.recip4, denom4)

            def epi_cast(st, on_scalar=False):
                st.outT_sb = epip.tile([128, QC], bf16, tag="outT_sb",
                                       name=f"outTs{st.c}")
                if on_scalar:
                    nc.scalar.copy(st.outT_sb, st.outT_ps)
                else:
                    nc.vector.tensor_copy(st.outT_sb, st.outT_ps)

            def epi_den_copy(st):
                st.den_sb = epip.tile([1, QC], bf16, tag="den_sb",
                                      name=f"dens{st.c}")
                nc.vector.tensor_copy(st.den_sb, st.den_ps)

            def epi_den4(st):
                # (128, 8) denominator halves: cols 0-3 from the den row via
                # K=1 extracts, cols 4-7 from the final batch's tile via
                # transposed ones-matmuls (every column is a self-contained
                # single-matmul group); one DVE reduce folds the halves,
                # then one reciprocal
                den4_ps = miscps.tile([128, 8], f32, tag="misc",
                                      name=f"den4{st.c}")
                e = st.stashed[st.nb - 1]
                for s in range(4):
                    nc.tensor.matmul(
                        den4_ps[:, 4 + s:5 + s],
                        lhsT=e[:, s * 128:(s + 1) * 128],
                        rhs=ones_col,
                        start=True,
                        stop=True,
                    )
                for s in range(4):
                    nc.tensor.matmul(
                        den4_ps[:, s:s + 1],
                        lhsT=st.den_sb[0:1, s * 128:(s + 1) * 128],
                        rhs=ones_1,
                        start=True,
                        stop=True,
                    )
                denom4 = epip.tile([128, 4], f32, tag="denom4",
                                   name=f"den4s{st.c}")
                nc.vector.tensor_reduce(
                    denom4,
                    den4_ps.rearrange("p (g s) -> p s g", g=2),
                    axis=mybir.AxisListType.X,
                    op=mybir.AluOpType.add,
                )
                st.recip4 = epip.tile([128, 4], f32, tag="recip4",
                                      name=f"rec{st.c}")
                nc.vector.reciprocal(st.recip4, denom4)

            def epi_outT_transpose(st, split=False):
                if split:
                    qa = miscps.tile([128, 256], bf16, tag="misc",
                                     name=f"outQa{st.c}")
                    qb = outps.tile([128, 256], bf16, tag="outT",
                                    name=f"outQb{st.c}")
                else:
                    qa = miscps.tile([128, QC], bf16, tag="misc",
                                     name=f"outQ{st.c}")
                    qb = None
                # (tile, col-block offset of q-block 2*half) per half
                st.outQ_parts = ((qa, 0), (qb, 0)) if split \
                    else ((qa, 0), (qa, 2))
                for s in range(4):
                    tgt, off = ((qa, s) if (not split or s < 2)
                                else (qb, s - 2))
                    nc.tensor.transpose(
                        tgt[:, off * 128:(off + 1) * 128],
                        st.outT_sb[:, s * 128:(s + 1) * 128],
                        ident_bf,
                    )
                # separate tiles per output half so the DVE and Scalar
                # scale pairs carry no false tile-level dependency
                st.out_sb = [
                    epip.tile([128, 2, 128], f32, tag=f"out_sb{h}",
                              name=f"outs{st.c}_{h}")
                    for h in range(2)
                ]

            def epi_scale_dma(st, half, on_scalar=False):
                c = st.c
                ob = st.out_sb[half]
                qt, base = st.outQ_parts[half]
                for j, s in enumerate((2 * half, 2 * half + 1)):
                    qs = qt[:, (base + j) * 128:(base + j + 1) * 128]
                    if on_scalar:
                        nc.scalar.mul(ob[:, j, :], qs,
                                      st.recip4[:, s:s + 1])
                    else:
                        nc.vector.tensor_scalar_mul(ob[:, j, :], qs,
                                                    st.recip4[:, s:s + 1])
                nc.sync.dma_start(
                    out=out_ext[c * QC + half * 256:c * QC + (half + 1) * 256,
                                :].rearrange("(s i) j -> i s j", s=2),
                    in_=ob,
                )

            # ---- software-pipelined chunk schedule --------------------
            st = start_chunk(0)
            emit_batch(st, 0)
            emit_batch(st, 1, flush=False)
            emit_batch(st, 2, flush=False)
            # lag-2 mm2 flushing while the vt stream catches up; catch-up
            # flushes mid-chunk shrink the boundary backlog to one batch
            for bi in range(3, st.nb - 1):
                emit_batch(st, bi,
                           flush_count=2 if bi in (12, 16, 20, 21) else 1)
            emit_batch(st, st.nb - 1, flush=False)
            st1 = start_chunk(1)
            emit_batch(st1, 0, flush=False)
            emit_batch(st1, 1, flush=False)
            flush_mm2(st)          # chunk 0's mm2 backlog
            epi_cast(st)
            emit_batch(st1, 2, flush=False)
            epi_fold0(st, 0)       # chunk-0 denominator on the DVE,
            emit_batch(st1, 3, flush=False)  # spread over c1 batches
            epi_fold0(st, 1)
            flush_mm2(st1, 2)      # mm2(c1 b0), mm2(c1 b1)
            emit_batch(st1, 4, flush_count=2)
            emit_batch(st1, 5)     # cadence restored
            epi_denom0(st)
            epi_outT_transpose(st)
            emit_batch(st1, 6)
            emit_batch(st1, 7)
            epi_scale_dma(st, 0)
            emit_batch(st1, 8)
            epi_scale_dma(st, 1)
            for bi in range(9, st1.nb):
                emit_batch(st1, bi)
            # final tail (cast on the now-idle Scalar engine)
            epi_cast(st1, on_scalar=True)
            epi_den4(st1)
            epi_outT_transpose(st1, split=True)
            epi_scale_dma(st1, 0)
            epi_scale_dma(st1, 1, on_scalar=True)
    return nc


def kernel(q, k, v):
    global LAST_RESULTS
    from concourse.bass_utils import run_bass_kernel_spmd

    q = np.ascontiguousarray(np.asarray(q, dtype=np.float32))
    k = np.ascontiguousarray(np.asarray(k, dtype=np.float32))
    v = np.ascontiguousarray(np.asarray(v, dtype=np.float32))

    # host-side layout prep: vt[p, 128t+c] = v[c, 128t+p] -- the exact SBUF
    # layout mm2 wants for its stationary operand (zero device transposes).
    vt = np.ascontiguousarray(
        v.reshape(D, NKV, KVT).transpose(2, 1, 0).reshape(D, SKV)
    )

    nc = _build_nc()
    nc.finalize()
    in_maps = [
        {
            "q": np.ascontiguousarray(q[:, i * SQS:(i + 1) * SQS]),
            "k": k,
            "vt": vt,
        }
        for i in range(NCORES)
    ]
    res = run_bass_kernel_spmd(nc, in_maps, core_ids=list(range(NCORES)))
    LAST_RESULTS = res
    out = np.concatenate([res.results[i]["out"] for i in range(NCORES)], axis=0)
    return out.astype(np.float32)


# revision 49
# speedup vs baseline: 1.0136x; 1.0136x over previous
"""Distributed manual-attention kernel for Trainium2 (8 NeuronCores).

Problem: q,k,v (128, 8192) f32; out = softmax(q^T k, axis=kv) @ v^T -> (8192, 128).

Strategy: shard seqlen_q across the 8 cores (1024 q columns each); k/v are
replicated.  Each core runs an independent flash-attention-style kernel:

  for each q-chunk (512 q):
    for each kv batch b (up to 3 tiles of 128 kv):
      S^T[b]   = k_tile^T @ q_chunk          (PE, bf16, out (kv, q) f32 PSUM)
      E[b]     = exp(S^T[b] - 60)            (ACT, bf16 out, bias rides free affine)
      outT    += vT_tile^T @ E[b]            (PE, bf16, accum (d, q) f32 PSUM)
      chain[i] += E[b]                       (DVE, bf16 2x mode)
    den[q]    = ones-matmuls over chains + late tiles   (PE, f32 accum)
    out       = transpose(outT) * 1/den      (PE transpose + split-engine scale)

v is fed to the device PRE-TRANSPOSED on the host (vt[p, 128t+c] = v[c, 128t+p],
the exact SBUF layout mm2's stationary operand wants): zero device transposes
of v.  Inputs arrive as bf16 via gpsimd-initiated CASTING DMAs (f32 HBM ->
bf16 SBUF in flight); q0 and two early k pieces ride the HWDGE path (sync
queue, f32 + DVE cast) in parallel with the gpsimd stream, halving the
staircase's delivery latency.

ACT (exp) is the pacing engine: 65536 exp elems per partition per core at
~1 elem/cycle.  Everything else hides underneath it:
  - chunk 0 staircases in (1,1,2,2,3...) with k DMA pieces cut to match, so
    the exp stream starts early and stays dense while the software DMA
    engine's ~3us transfer latency catches up; mm2 batches run at lag-2
    early on so a late vt piece never head-of-line blocks an mm1.
  - the denominator engine is chosen per chunk by which engine has slack
    where that chunk's epilogue lands.  Chunk 0's epilogue overlaps chunk
    1's steady state, where the DVE idles ~0.8us/window but the PE only
    ~0.25us: its chain folds on the DVE (adds -> PE transpose -> reduce ->
    reciprocal).  Chunk 1's epilogue IS the tail, where latency rules: its
    chain collapses via accumulating ones-matmuls (ones^T @ acc / E) into
    a (1,512) PSUM row closed before the last exp, extracted to
    q-partitions by four K=1 matmuls; the final 1-tile batch enters the
    extract tile directly via transposed ones-matmuls (E_slice^T @ ones),
    one DVE reduce folds the halves -- after the last exp only ~1us of
    work gates the reciprocal.
  - at the chunk boundary both chunks' mm2 backlogs are deferred behind
    chunk 1's early mm1s; chunk 1 opens with a 1-tile batch.
  - the tail splits across engines: den-copy + cast + recip + 2 scales on
    DVE, 2 scales on Scalar, den/extract/transpose matmuls on PE, with
    each output DMA issued right after its two scales.

exp is computed as exp(qk - 60): softmax is shift-invariant and row maxima
of qk reach ~117 > ln(f32_max)=88.7, so unshifted exp overflows f32 on ~2%
of rows.  With the shift, exp <= e^57: safe in f32 and bf16.
"""

import numpy as np

D = 128          # head dim
SQ = 8192        # total seqlen_q
SKV = 8192       # seqlen_kv
NCORES = 8
SQS = SQ // NCORES   # 1024 q per core
QC = 512             # q chunk (matmul moving free dim)
NQC = SQS // QC      # 2 chunks
KVT = 128            # kv tile (PE contraction / partition dim)
NKV = SKV // KVT     # 64 kv tiles
N_WARMUP = 4         # PE warm-up matmuls (HAM ramp)

# kv-tile batch sizes per chunk (sum = NKV).
BATCHES_C0 = [1, 1, 2, 2] + [3] * 19 + [1]
BATCHES_C1 = [1] + [3] * 20 + [2, 1]

# k DMA pieces (col ranges) cut to the chunk-0 consumption staircase.
K_PIECES = [(0, 128), (128, 256), (256, 512), (512, 768), (768, 1152),
            (1152, 1536)] + [
    (1536 + 512 * i, 1536 + 512 * (i + 1)) for i in range(13)
]
VT_PIECES = [(512 * i, 512 * (i + 1)) for i in range(16)]

LAST_RESULTS = None  # BassKernelResults of the most recent run (for test.py)


def _build_nc():
    import concourse.tile as tile
    from concourse import bacc, mybir
    from concourse.masks import make_identity

    f32 = mybir.dt.float32
    bf16 = mybir.dt.bfloat16

    nc = bacc.Bacc(None, target_bir_lowering=False)
    q_ext = nc.declare_dram_parameter("q", [D, SQS], f32, isOutput=False)
    k_ext = nc.declare_dram_parameter("k", [D, SKV], f32, isOutput=False)
    vt_ext = nc.declare_dram_parameter("vt", [D, SKV], f32, isOutput=False)
    out_ext = nc.declare_dram_parameter("out", [SQS, D], f32, isOutput=True)

    def mk_batches(sizes):
        out, t = [], 0
        for s in sizes:
            out.append(list(range(t, t + s)))
            t += s
        assert t == NKV
        return out

    batches_by_chunk = [mk_batches(BATCHES_C0), mk_batches(BATCHES_C1)]

    with tile.TileContext(nc) as tc:
        with (
            tc.tile_pool(name="const", bufs=1) as constp,
            tc.tile_pool(name="inputs", bufs=1) as inputs,
            tc.tile_pool(name="work", bufs=7) as workp,
            tc.tile_pool(name="accp", bufs=2) as accp,
            tc.tile_pool(name="epi", bufs=2) as epip,
            tc.tile_pool(name="qk_ps", bufs=2, space="PSUM") as qkps,
            tc.tile_pool(name="out_ps", bufs=1, space="PSUM") as outps,
            tc.tile_pool(name="misc_ps", bufs=1, space="PSUM") as miscps,
        ):
            # ---- lead-in ----------------------------------------------
            k0_tile = inputs.tile([D, K_PIECES[0][1]], bf16, name="k0",
                                  tag="k0")
            nc.gpsimd.dma_start(out=k0_tile, in_=k_ext[:, 0:K_PIECES[0][1]])
            scratch = constp.tile([128, 512], bf16, name="scratch")
            nc.gpsimd.memset(scratch, 0.0)
            bias_m60 = constp.tile([128, 1], f32, name="bias_m60")
            nc.gpsimd.memset(bias_m60, -60.0)
            dummy = constp.tile([128, 1], f32, name="dummy")
            nc.scalar.activation(dummy, bias_m60,
                                 func=mybir.ActivationFunctionType.Exp)
            warm_ps = outps.tile([128, 512], f32, tag="outT", name="warm_ps")
            for _ in range(N_WARMUP):
                nc.tensor.matmul(
                    warm_ps, lhsT=scratch[:, 0:128], rhs=scratch,
                    start=True, stop=True,
                )

            q_tiles = [inputs.tile([D, QC], bf16, name=f"q{c}", tag=f"q{c}")
                       for c in range(NQC)]
            k_pieces = [k0_tile] + [
                inputs.tile([D, hi - lo], bf16, name=f"k{i}", tag=f"k{i}")
                for i, (lo, hi) in enumerate(K_PIECES) if i > 0
            ]
            vt_pieces = [
                inputs.tile([D, hi - lo], bf16, name=f"vt{i}", tag=f"vt{i}")
                for i, (lo, hi) in enumerate(VT_PIECES)
            ]

            # q0 and k1/k3 ride the HWDGE path (sync queue, f32 + DVE cast)
            # IN PARALLEL with k0/k2/k4 on the gpsimd casting queue: two DMA
            # paths halve the staircase's serialized delivery latency.
            q0_f32 = inputs.tile([D, QC], f32, name="q0f", tag="q0f")
            nc.sync.dma_start(out=q0_f32, in_=q_ext[:, 0:QC])
            nc.vector.tensor_copy(q_tiles[0], q0_f32)

            def dma_k(i):
                lo, hi = K_PIECES[i]
                nc.gpsimd.dma_start(out=k_pieces[i], in_=k_ext[:, lo:hi])

            def dma_k_sync(i):
                lo, hi = K_PIECES[i]
                kf = inputs.tile([D, hi - lo], f32, name=f"kf{i}",
                                 tag=f"kf{i}")
                nc.sync.dma_start(out=kf, in_=k_ext[:, lo:hi])
                nc.vector.tensor_copy(k_pieces[i], kf)

            def dma_vt(i):
                lo, hi = VT_PIECES[i]
                nc.gpsimd.dma_start(out=vt_pieces[i], in_=vt_ext[:, lo:hi])

            dma_k_sync(1)
            dma_k(2)
            dma_k_sync(3)
            dma_vt(0)
            dma_k(4)
            ki, vi = 5, 1
            for step in range(16):
                if vi < 16:
                    dma_vt(vi)
                    vi += 1
                if ki < len(K_PIECES):
                    dma_k(ki)
                    ki += 1
                if step == 4:
                    nc.gpsimd.dma_start(out=q_tiles[1],
                                        in_=q_ext[:, QC:2 * QC])
            assert ki == len(K_PIECES) and vi == 16

            # constants for the epilogue (needed only mid-kernel)
            ident_bf = constp.tile([128, 128], bf16, name="ident_bf")
            make_identity(nc, ident_bf)
            ones_col = constp.tile([128, 1], bf16, name="ones_col")
            nc.gpsimd.memset(ones_col, 1.0)
            ones_1 = constp.tile([1, 1], bf16, name="ones_1")
            nc.gpsimd.memset(ones_1, 1.0)

            # ---- lhsT lookups ------------------------------------------
            k_start = [lo for lo, _ in K_PIECES]

            def mm1_lhsT(t):
                col = t * KVT
                for i in range(len(K_PIECES) - 1, -1, -1):
                    if k_start[i] <= col:
                        off = col - k_start[i]
                        return k_pieces[i][:, off:off + KVT]
                raise AssertionError

            def mm2_lhsT(t):
                return vt_pieces[t // 4][:, (t % 4) * KVT:(t % 4) * KVT + KVT]

            # ---- per-chunk state ---------------------------------------
            class Chunk:
                pass

            def start_chunk(c):
                st = Chunk()
                st.c = c
                st.batches = batches_by_chunk[c]
                st.nb = len(st.batches)
                # last 2 batches bypass the chains; the final one bypasses
                # even the den row (transposed matmuls in the tail)
                st.direct = {st.nb - 2, st.nb - 1}
                chained = [b for b in range(st.nb) if b not in st.direct]
                st.chain_of = {b: 0 for b in chained}
                st.chain_prev = [None]
                st.chain_live = [False]
                st.chain_width = [0]
                st.chain_last = max(chained)
                st.q_rhs = q_tiles[c]
                st.outT_ps = outps.tile([128, QC], f32, tag="outT",
                                        name=f"outT{c}")
                st.accs = [
                    accp.tile([128, 3 * QC], bf16, tag="acc0",
                              name=f"acc{c}_0")
                ]
                st.stashed = {}
                st.mm2_pending = []
                # den row matmul count: the acc's slices + the
                # second-to-last batch's tiles (the final batch goes
                # straight into the extract tile via transposed matmuls)
                st.den_total = 3 + len(st.batches[st.nb - 2])
                st.den_emitted = 0
                st.den_ps = None
                return st

            def flush_mm2(st, count=None):
                n = len(st.mm2_pending) if count is None else count
                for batch, exp3 in st.mm2_pending[:n]:
                    for j, t in enumerate(batch):
                        nc.tensor.matmul(
                            st.outT_ps,
                            lhsT=mm2_lhsT(t),
                            rhs=exp3[:, j * QC:(j + 1) * QC],
                            start=(t == 0),
                            stop=(t == NKV - 1),
                        )
                del st.mm2_pending[:n]

            def den_mm(st, rhs512):
                # accumulate ones^T @ rhs into this chunk's den row (PE)
                if st.den_ps is None:
                    st.den_ps = miscps.tile([1, QC], f32, tag="misc",
                                            name=f"den{st.c}")
                nc.tensor.matmul(
                    st.den_ps, lhsT=ones_col, rhs=rhs512,
                    start=(st.den_emitted == 0),
                    stop=(st.den_emitted == st.den_total - 1),
                )
                st.den_emitted += 1

            def den_mm_acc(st, j):
                for sl in range(3):
                    den_mm(st, st.accs[j][:, sl * QC:(sl + 1) * QC])

            def den_mm_exp(st, bi):
                e = st.stashed[bi]
                for sl in range(len(st.batches[bi])):
                    den_mm(st, e[:, sl * QC:(sl + 1) * QC])

            def emit_chain(st, bi, exp3, w):
                # A narrower tile adds into the accumulator's low columns
                # only (all columns are summed by the den matmuls anyway);
                # a wider tile extends the accumulator's live width.
                ch = st.chain_of[bi]
                acc = st.accs[ch]
                if st.chain_prev[ch] is None and not st.chain_live[ch]:
                    st.chain_prev[ch] = (exp3, w)
                    return
                if not st.chain_live[ch]:
                    pexp, pw = st.chain_prev[ch]
                    lo = min(pw, w)
                    nc.vector.tensor_add(acc[:, :lo], pexp[:, :lo],
                                         exp3[:, :lo])
                    if w > pw:
                        nc.vector.tensor_copy(acc[:, pw:w], exp3[:, pw:w])
                    elif pw > w:
                        nc.vector.tensor_copy(acc[:, w:pw], pexp[:, w:pw])
                    st.chain_prev[ch] = None
                    st.chain_live[ch] = True
                    st.chain_width[ch] = max(pw, w)
                    return
                cw = st.chain_width[ch]
                lo = min(cw, w)
                nc.vector.tensor_add(acc[:, :lo], acc[:, :lo], exp3[:, :lo])
                if w > cw:
                    nc.vector.tensor_copy(acc[:, cw:w], exp3[:, cw:w])
                    st.chain_width[ch] = w

            def emit_batch(st, bi, flush=True, flush_count=None):
                c = st.c
                batch = st.batches[bi]
                w = len(batch) * QC
                qk_ps = qkps.tile([128, 3 * QC], f32, tag="qk",
                                  name=f"qk{c}_{bi}")
                for j, t in enumerate(batch):
                    nc.tensor.matmul(
                        qk_ps[:, j * QC:(j + 1) * QC],
                        lhsT=mm1_lhsT(t),
                        rhs=st.q_rhs,
                        start=True,
                        stop=True,
                    )
                exp3 = workp.tile([128, 3 * QC], bf16, tag="exp3",
                                  name=f"exp{c}_{bi}")
                nc.scalar.activation(
                    exp3[:, :w], qk_ps[:, :w],
                    func=mybir.ActivationFunctionType.Exp,
                    bias=bias_m60,
                )
                if flush:
                    flush_mm2(st, flush_count)
                if bi in st.direct:
                    st.stashed[bi] = exp3
                else:
                    emit_chain(st, bi, exp3, w)
                # dribbled den-row matmuls, each placed one batch after its
                # source engine finished so the PE queue never waits on them
                if bi == st.chain_last + 2 and st.c == NQC - 1:
                    den_mm_acc(st, 0)
                if bi == st.nb - 1 and st.c == NQC - 1:
                    # the pre-final direct batch's row matmuls: its exp is
                    # done, so the row closes during this (last) exp
                    den_mm_exp(st, st.nb - 2)
                if bi == st.nb - 1 and st.den_emitted == st.den_total:
                    # den row complete: pull it to SBUF on the Scalar queue
                    # (idle after this exp) so it overlaps the DVE cast
                    epi_den_copy(st, on_scalar=True)
                st.mm2_pending.append((batch, exp3))
                if bi == st.nb - 1 and flush:
                    flush_mm2(st)
                    if c < NQC - 1:
                        epi_cast(st)

            # ---- epilogue stages ---------------------------------------
            def epi_fold0(st, part):
                # chunk-0 denominator on the DVE (its epilogue overlaps
                # chunk 1, where the DVE has slack and the PE does not):
                # fold the chain + direct tiles to a 512-wide acc_sum
                acc = st.accs[0]
                if part == 0:
                    st.acc_sum = epip.tile([128, QC], bf16, tag="acc_sum",
                                           name=f"accs{st.c}")
                    nc.vector.tensor_add(st.acc_sum, acc[:, 0:QC],
                                         acc[:, QC:2 * QC])
                    nc.vector.tensor_add(st.acc_sum, st.acc_sum,
                                         acc[:, 2 * QC:3 * QC])
                else:
                    e2 = st.stashed[st.nb - 2]
                    for sl in range(len(st.batches[st.nb - 2])):
                        nc.vector.tensor_add(st.acc_sum, st.acc_sum,
                                             e2[:, sl * QC:(sl + 1) * QC])
                    nc.vector.tensor_add(st.acc_sum, st.acc_sum,
                                         st.stashed[st.nb - 1][:, 0:QC])

            def epi_denom0(st):
                accT_ps = miscps.tile([128, QC], bf16, tag="misc",
                                      name=f"accT{st.c}")
                for s in range(4):
                    nc.tensor.transpose(
                        accT_ps[:, s * 128:(s + 1) * 128],
                        st.acc_sum[:, s * 128:(s + 1) * 128],
                        ident_bf,
                    )
                denom4 = epip.tile([128, 4], f32, tag="denom4",
                                   name=f"den4s{st.c}")
                nc.vector.tensor_reduce(
                    denom4,
                    accT_ps.rearrange("p (s j) -> p s j", s=4),
                    axis=mybir.AxisListType.X,
                    op=mybir.AluOpType.add,
                )
                st.recip4 = epip.tile([128, 4], f32, tag="recip4",
                                      name=f"rec{st.c}")
                nc.vector.reciprocal(st.recip4, denom4)

            def epi_cast(st, on_scalar=False):
                st.outT_sb = epip.tile([128, QC], bf16, tag="outT_sb",
                                       name=f"outTs{st.c}")
                if on_scalar:
                    nc.scalar.copy(st.outT_sb, st.outT_ps)
                else:
                    nc.vector.tensor_copy(st.outT_sb, st.outT_ps)

            def epi_den_copy(st, on_scalar=False):
                st.den_sb = epip.tile([1, QC], bf16, tag="den_sb",
                                      name=f"dens{st.c}")
                if on_scalar:
                    nc.scalar.copy(st.den_sb, st.den_ps)
                else:
                    nc.vector.tensor_copy(st.den_sb, st.den_ps)

            def epi_den4(st):
                # (128, 8) denominator halves: cols 0-3 from the den row via
                # K=1 extracts, cols 4-7 from the final batch's tile via
                # transposed ones-matmuls (every column is a self-contained
                # single-matmul group); one DVE reduce folds the halves,
                # then one reciprocal
                den4_ps = miscps.tile([128, 8], f32, tag="misc",
                                      name=f"den4{st.c}")
                e = st.stashed[st.nb - 1]
                for s in range(4):
                    nc.tensor.matmul(
                        den4_ps[:, 4 + s:5 + s],
                        lhsT=e[:, s * 128:(s + 1) * 128],
                        rhs=ones_col,
                        start=True,
                        stop=True,
                    )
                for s in range(4):
                    nc.tensor.matmul(
                        den4_ps[:, s:s + 1],
                        lhsT=st.den_sb[0:1, s * 128:(s + 1) * 128],
                        rhs=ones_1,
                        start=True,
                        stop=True,
                    )
                denom4 = epip.tile([128, 4], f32, tag="denom4",
                                   name=f"den4s{st.c}")
                nc.vector.tensor_reduce(
                    denom4,
                    den4_ps.rearrange("p (g s) -> p s g", g=2),
                    axis=mybir.AxisListType.X,
                    op=mybir.AluOpType.add,
                )
                st.recip4 = epip.tile([128, 4], f32, tag="recip4",
                                      name=f"rec{st.c}")
                nc.vector.reciprocal(st.recip4, denom4)

            def epi_outT_transpose(st, split=False):
                if split:
                    qa = miscps.tile([128, 256], bf16, tag="misc",
                                     name=f"outQa{st.c}")
                    qb = outps.tile([128, 256], bf16, tag="outT",
                                    name=f"outQb{st.c}")
                else:
                    qa = miscps.tile([128, QC], bf16, tag="misc",
                                     name=f"outQ{st.c}")
                    qb = None
                # (tile, col-block offset of q-block 2*half) per half
                st.outQ_parts = ((qa, 0), (qb, 0)) if split \
                    else ((qa, 0), (qa, 2))
                for s in range(4):
                    tgt, off = ((qa, s) if (not split or s < 2)
                                else (qb, s - 2))
                    nc.tensor.transpose(
                        tgt[:, off * 128:(off + 1) * 128],
                        st.outT_sb[:, s * 128:(s + 1) * 128],
                        ident_bf,
                    )
                # separate tiles per output half so the DVE and Scalar
                # scale pairs carry no false tile-level dependency
                st.out_sb = [
                    epip.tile([128, 2, 128], f32, tag=f"out_sb{h}",
                              name=f"outs{st.c}_{h}")
                    for h in range(2)
                ]

            def epi_scale_dma(st, half, on_scalar=False):
                c = st.c
                ob = st.out_sb[half]
                qt, base = st.outQ_parts[half]
                for j, s in enumerate((2 * half, 2 * half + 1)):
                    qs = qt[:, (base + j) * 128:(base + j + 1) * 128]
                    if on_scalar:
                        nc.scalar.mul(ob[:, j, :], qs,
                                      st.recip4[:, s:s + 1])
                    else:
                        nc.vector.tensor_scalar_mul(ob[:, j, :], qs,
                                                    st.recip4[:, s:s + 1])
                nc.sync.dma_start(
                    out=out_ext[c * QC + half * 256:c * QC + (half + 1) * 256,
                                :].rearrange("(s i) j -> i s j", s=2),
                    in_=ob,
                )

            # ---- software-pipelined chunk schedule --------------------
            st = start_chunk(0)
            emit_batch(st, 0)
            emit_batch(st, 1, flush=False)
            emit_batch(st, 2, flush=False)
            # lag-2 mm2 flushing while the vt stream catches up; catch-up
            # flushes mid-chunk shrink the boundary backlog to one batch
            for bi in range(3, st.nb - 1):
                emit_batch(st, bi,
                           flush_count=2 if bi in (12, 16, 20, 21) else 1)
            emit_batch(st, st.nb - 1, flush=False)
            st1 = start_chunk(1)
            emit_batch(st1, 0, flush=False)
            emit_batch(st1, 1, flush=False)
            flush_mm2(st)          # chunk 0's mm2 backlog
            epi_cast(st)
            emit_batch(st1, 2, flush=False)
            epi_fold0(st, 0)       # chunk-0 denominator on the DVE,
            emit_batch(st1, 3, flush=False)  # spread over c1 batches
            epi_fold0(st, 1)
            flush_mm2(st1, 2)      # mm2(c1 b0), mm2(c1 b1)
            emit_batch(st1, 4, flush_count=2)
            emit_batch(st1, 5)     # cadence restored
            epi_denom0(st)
            epi_outT_transpose(st)
            emit_batch(st1, 6)
            emit_batch(st1, 7)
            epi_scale_dma(st, 0)
            emit_batch(st1, 8)
            epi_scale_dma(st, 1)
            for bi in range(9, st1.nb):
                emit_batch(st1, bi)
            # final tail (den-copy holds the Scalar queue, cast on DVE)
            epi_cast(st1)
            epi_den4(st1)
            epi_outT_transpose(st1, split=True)
            epi_scale_dma(st1, 0)
            epi_scale_dma(st1, 1, on_scalar=True)
    return nc


def kernel(q, k, v):
    global LAST_RESULTS
    from concourse.bass_utils import run_bass_kernel_spmd

    q = np.ascontiguousarray(np.asarray(q, dtype=np.float32))
    k = np.ascontiguousarray(np.asarray(k, dtype=np.float32))
    v = np.ascontiguousarray(np.asarray(v, dtype=np.float32))

    # host-side layout prep: vt[p, 128t+c] = v[c, 128t+p] -- the exact SBUF
    # layout mm2 wants for its stationary operand (zero device transposes).
    vt = np.ascontiguousarray(
        v.reshape(D, NKV, KVT).transpose(2, 1, 0).reshape(D, SKV)
    )

    nc = _build_nc()
    nc.finalize()
    in_maps = [
        {
            "q": np.ascontiguousarray(q[:, i * SQS:(i + 1) * SQS]),
            "k": k,
            "vt": vt,
        }
        for i in range(NCORES)
    ]
    res = run_bass_kernel_spmd(nc, in_maps, core_ids=list(range(NCORES)))
    LAST_RESULTS = res
    out = np.concatenate([res.results[i]["out"] for i in range(NCORES)], axis=0)
    return out.astype(np.float32)


# revision 51
# speedup vs baseline: 1.0294x; 1.0156x over previous
"""Distributed manual-attention kernel for Trainium2 (8 NeuronCores).

Problem: q,k,v (128, 8192) f32; out = softmax(q^T k, axis=kv) @ v^T -> (8192, 128).

Strategy: shard seqlen_q across the 8 cores (1024 q columns each); k/v are
replicated.  Each core runs an independent flash-attention-style kernel:

  for each q-chunk (512 q):
    for each kv batch b (up to 3 tiles of 128 kv):
      S^T[b]   = k_tile^T @ q_chunk          (PE, bf16, out (kv, q) f32 PSUM)
      E[b]     = exp(S^T[b] - 60)            (ACT, bf16 out, bias rides free affine)
      outT    += vT_tile^T @ E[b]            (PE, bf16, accum (d, q) f32 PSUM)
      chain[i] += E[b]                       (DVE, bf16 2x mode)
    den[q]    = ones-matmuls over chains + late tiles   (PE, f32 accum)
    out       = transpose(outT) * 1/den      (PE transpose + split-engine scale)

v is fed to the device PRE-TRANSPOSED on the host (vt[p, 128t+c] = v[c, 128t+p],
the exact SBUF layout mm2's stationary operand wants): zero device transposes
of v.  Inputs arrive as bf16 via gpsimd-initiated CASTING DMAs (f32 HBM ->
bf16 SBUF in flight); q0 and two early k pieces ride the HWDGE path (sync
queue, f32 + DVE cast) in parallel with the gpsimd stream, halving the
staircase's delivery latency.

ACT (exp) is the pacing engine: 65536 exp elems per partition per core at
~1 elem/cycle.  Everything else hides underneath it:
  - chunk 0 staircases in (1,1,2,2,3...) with k DMA pieces cut to match, so
    the exp stream starts early and stays dense while the software DMA
    engine's ~3us transfer latency catches up; mm2 batches run at lag-2
    early on so a late vt piece never head-of-line blocks an mm1.
  - the denominator engine is chosen per chunk by which engine has slack
    where that chunk's epilogue lands.  Chunk 0's epilogue overlaps chunk
    1's steady state, where the DVE idles ~0.8us/window but the PE only
    ~0.25us: its chain folds on the DVE (adds -> PE transpose -> reduce ->
    reciprocal).  Chunk 1's epilogue IS the tail, where latency rules: its
    chain collapses via accumulating ones-matmuls (ones^T @ acc / E) into
    a (1,512) PSUM row closed before the last exp, extracted to
    q-partitions by four K=1 matmuls; the final 1-tile batch enters the
    extract tile directly via transposed ones-matmuls (E_slice^T @ ones),
    one DVE reduce folds the halves -- after the last exp only ~1us of
    work gates the reciprocal.
  - at the chunk boundary both chunks' mm2 backlogs are deferred behind
    chunk 1's early mm1s; chunk 1 opens with a 1-tile batch.
  - the tail splits across engines: den-copy + cast + recip + 2 scales on
    DVE, 2 scales on Scalar, den/extract/transpose matmuls on PE, with
    each output DMA issued right after its two scales.

exp is computed as exp(qk - 60): softmax is shift-invariant and row maxima
of qk reach ~117 > ln(f32_max)=88.7, so unshifted exp overflows f32 on ~2%
of rows.  With the shift, exp <= e^57: safe in f32 and bf16.
"""

import numpy as np

D = 128          # head dim
SQ = 8192        # total seqlen_q
SKV = 8192       # seqlen_kv
NCORES = 8
SQS = SQ // NCORES   # 1024 q per core
QC = 512             # q chunk (matmul moving free dim)
NQC = SQS // QC      # 2 chunks
KVT = 128            # kv tile (PE contraction / partition dim)
NKV = SKV // KVT     # 64 kv tiles
N_WARMUP = 4         # PE warm-up matmuls (HAM ramp)

# kv-tile batch sizes per chunk (sum = NKV).
BATCHES_C0 = [1, 1, 2, 2] + [3] * 19 + [1]
BATCHES_C1 = [1] + [3] * 20 + [2, 1]

# k DMA pieces (col ranges) cut to the chunk-0 consumption staircase.
K_PIECES = [(0, 128), (128, 256), (256, 512), (512, 768), (768, 1152),
            (1152, 1536)] + [
    (1536 + 512 * i, 1536 + 512 * (i + 1)) for i in range(13)
]
VT_PIECES = [(512 * i, 512 * (i + 1)) for i in range(16)]

LAST_RESULTS = None  # BassKernelResults of the most recent run (for test.py)


def _build_nc():
    import concourse.tile as tile
    from concourse import bacc, mybir
    from concourse.masks import make_identity

    f32 = mybir.dt.float32
    bf16 = mybir.dt.bfloat16

    nc = bacc.Bacc(None, target_bir_lowering=False)
    q_ext = nc.declare_dram_parameter("q", [D, SQS], f32, isOutput=False)
    k_ext = nc.declare_dram_parameter("k", [D, SKV], f32, isOutput=False)
    vt_ext = nc.declare_dram_parameter("vt", [D, SKV], f32, isOutput=False)
    out_ext = nc.declare_dram_parameter("out", [SQS, D], f32, isOutput=True)

    def mk_batches(sizes):
        out, t = [], 0
        for s in sizes:
            out.append(list(range(t, t + s)))
            t += s
        assert t == NKV
        return out

    batches_by_chunk = [mk_batches(BATCHES_C0), mk_batches(BATCHES_C1)]

    with tile.TileContext(nc) as tc:
        with (
            tc.tile_pool(name="const", bufs=1) as constp,
            tc.tile_pool(name="inputs", bufs=1) as inputs,
            tc.tile_pool(name="work", bufs=7) as workp,
            tc.tile_pool(name="accp", bufs=2) as accp,
            tc.tile_pool(name="epi", bufs=2) as epip,
            tc.tile_pool(name="qk_ps", bufs=2, space="PSUM") as qkps,
            tc.tile_pool(name="out_ps", bufs=1, space="PSUM") as outps,
            tc.tile_pool(name="misc_ps", bufs=1, space="PSUM") as miscps,
        ):
            # ---- lead-in ----------------------------------------------
            k0_tile = inputs.tile([D, K_PIECES[0][1]], bf16, name="k0",
                                  tag="k0")
            nc.gpsimd.dma_start(out=k0_tile, in_=k_ext[:, 0:K_PIECES[0][1]])
            scratch = constp.tile([128, 512], bf16, name="scratch")
            nc.gpsimd.memset(scratch, 0.0)
            bias_m60 = constp.tile([128, 1], f32, name="bias_m60")
            nc.gpsimd.memset(bias_m60, -60.0)
            dummy = constp.tile([128, 1], f32, name="dummy")
            nc.scalar.activation(dummy, bias_m60,
                                 func=mybir.ActivationFunctionType.Exp)
            warm_ps = outps.tile([128, 512], f32, tag="outT", name="warm_ps")
            for _ in range(N_WARMUP):
                nc.tensor.matmul(
                    warm_ps, lhsT=scratch[:, 0:128], rhs=scratch,
                    start=True, stop=True,
                )

            q_tiles = [inputs.tile([D, QC], bf16, name=f"q{c}", tag=f"q{c}")
                       for c in range(NQC)]
            k_pieces = [k0_tile] + [
                inputs.tile([D, hi - lo], bf16, name=f"k{i}", tag=f"k{i}")
                for i, (lo, hi) in enumerate(K_PIECES) if i > 0
            ]
            vt_pieces = [
                inputs.tile([D, hi - lo], bf16, name=f"vt{i}", tag=f"vt{i}")
                for i, (lo, hi) in enumerate(VT_PIECES)
            ]

            # q0 and k1/k3 ride the HWDGE path (sync queue, f32 + DVE cast)
            # IN PARALLEL with k0/k2/k4 on the gpsimd casting queue: two DMA
            # paths halve the staircase's serialized delivery latency.
            q0_f32 = inputs.tile([D, QC], f32, name="q0f", tag="q0f")
            nc.sync.dma_start(out=q0_f32, in_=q_ext[:, 0:QC])
            nc.vector.tensor_copy(q_tiles[0], q0_f32)

            def dma_k(i):
                lo, hi = K_PIECES[i]
                nc.gpsimd.dma_start(out=k_pieces[i], in_=k_ext[:, lo:hi])

            def dma_k_sync(i):
                lo, hi = K_PIECES[i]
                kf = inputs.tile([D, hi - lo], f32, name=f"kf{i}",
                                 tag=f"kf{i}")
                nc.sync.dma_start(out=kf, in_=k_ext[:, lo:hi])
                nc.vector.tensor_copy(k_pieces[i], kf)

            def dma_vt(i):
                lo, hi = VT_PIECES[i]
                nc.gpsimd.dma_start(out=vt_pieces[i], in_=vt_ext[:, lo:hi])

            dma_k_sync(1)
            dma_k(2)
            dma_k_sync(3)
            dma_vt(0)
            dma_k(4)
            ki, vi = 5, 1
            for step in range(16):
                if vi < 16:
                    dma_vt(vi)
                    vi += 1
                if ki < len(K_PIECES):
                    dma_k(ki)
                    ki += 1
                if step == 4:
                    nc.gpsimd.dma_start(out=q_tiles[1],
                                        in_=q_ext[:, QC:2 * QC])
            assert ki == len(K_PIECES) and vi == 16

            # constants for the epilogue (needed only mid-kernel)
            ident_bf = constp.tile([128, 128], bf16, name="ident_bf")
            make_identity(nc, ident_bf)
            ones_col = constp.tile([128, 1], bf16, name="ones_col")
            nc.gpsimd.memset(ones_col, 1.0)
            ones_1 = constp.tile([1, 1], bf16, name="ones_1")
            nc.gpsimd.memset(ones_1, 1.0)

            # ---- lhsT lookups ------------------------------------------
            k_start = [lo for lo, _ in K_PIECES]

            def mm1_lhsT(t):
                col = t * KVT
                for i in range(len(K_PIECES) - 1, -1, -1):
                    if k_start[i] <= col:
                        off = col - k_start[i]
                        return k_pieces[i][:, off:off + KVT]
                raise AssertionError

            def mm2_lhsT(t):
                return vt_pieces[t // 4][:, (t % 4) * KVT:(t % 4) * KVT + KVT]

            # ---- per-chunk state ---------------------------------------
            class Chunk:
                pass

            def start_chunk(c):
                st = Chunk()
                st.c = c
                st.batches = batches_by_chunk[c]
                st.nb = len(st.batches)
                # last 2 batches bypass the chains; the final one bypasses
                # even the den row (transposed matmuls in the tail)
                st.direct = {st.nb - 2, st.nb - 1}
                chained = [b for b in range(st.nb) if b not in st.direct]
                st.chain_of = {b: 0 for b in chained}
                st.chain_prev = [None]
                st.chain_live = [False]
                st.chain_width = [0]
                st.chain_last = max(chained)
                st.q_rhs = q_tiles[c]
                st.outT_ps = outps.tile([128, QC], f32, tag="outT",
                                        name=f"outT{c}")
                st.accs = [
                    accp.tile([128, 3 * QC], bf16, tag="acc0",
                              name=f"acc{c}_0")
                ]
                st.stashed = {}
                st.mm2_pending = []
                # den row matmul count: the acc's slices + the
                # second-to-last batch's tiles (the final batch goes
                # straight into the extract tile via transposed matmuls)
                st.den_total = 3 + len(st.batches[st.nb - 2])
                st.den_emitted = 0
                st.den_ps = None
                return st

            def flush_mm2(st, count=None):
                n = len(st.mm2_pending) if count is None else count
                for batch, exp3 in st.mm2_pending[:n]:
                    for j, t in enumerate(batch):
                        nc.tensor.matmul(
                            st.outT_ps,
                            lhsT=mm2_lhsT(t),
                            rhs=exp3[:, j * QC:(j + 1) * QC],
                            start=(t == 0),
                            stop=(t == NKV - 1),
                        )
                del st.mm2_pending[:n]

            def den_mm(st, rhs512):
                # accumulate ones^T @ rhs into this chunk's den row (PE)
                if st.den_ps is None:
                    st.den_ps = miscps.tile([1, QC], f32, tag="misc",
                                            name=f"den{st.c}")
                nc.tensor.matmul(
                    st.den_ps, lhsT=ones_col, rhs=rhs512,
                    start=(st.den_emitted == 0),
                    stop=(st.den_emitted == st.den_total - 1),
                )
                st.den_emitted += 1

            def den_mm_acc(st, j):
                for sl in range(3):
                    den_mm(st, st.accs[j][:, sl * QC:(sl + 1) * QC])

            def den_mm_exp(st, bi):
                e = st.stashed[bi]
                for sl in range(len(st.batches[bi])):
                    den_mm(st, e[:, sl * QC:(sl + 1) * QC])

            def emit_chain(st, bi, exp3, w):
                # A narrower tile adds into the accumulator's low columns
                # only (all columns are summed by the den matmuls anyway);
                # a wider tile extends the accumulator's live width.
                ch = st.chain_of[bi]
                acc = st.accs[ch]
                if st.chain_prev[ch] is None and not st.chain_live[ch]:
                    st.chain_prev[ch] = (exp3, w)
                    return
                if not st.chain_live[ch]:
                    pexp, pw = st.chain_prev[ch]
                    lo = min(pw, w)
                    nc.vector.tensor_add(acc[:, :lo], pexp[:, :lo],
                                         exp3[:, :lo])
                    if w > pw:
                        nc.vector.tensor_copy(acc[:, pw:w], exp3[:, pw:w])
                    elif pw > w:
                        nc.vector.tensor_copy(acc[:, w:pw], pexp[:, w:pw])
                    st.chain_prev[ch] = None
                    st.chain_live[ch] = True
                    st.chain_width[ch] = max(pw, w)
                    return
                cw = st.chain_width[ch]
                lo = min(cw, w)
                nc.vector.tensor_add(acc[:, :lo], acc[:, :lo], exp3[:, :lo])
                if w > cw:
                    nc.vector.tensor_copy(acc[:, cw:w], exp3[:, cw:w])
                    st.chain_width[ch] = w

            def emit_batch(st, bi, flush=True, flush_count=None):
                c = st.c
                batch = st.batches[bi]
                w = len(batch) * QC
                if bi == st.chain_last + 2 and st.c == NQC - 1:
                    # den-row acc matmuls at the HEAD of this emission:
                    # their input closed a window ago, and the mm1 behind
                    # them still makes its window -- this lets the den row
                    # finish before the last exp
                    den_mm_acc(st, 0)
                qk_ps = qkps.tile([128, 3 * QC], f32, tag="qk",
                                  name=f"qk{c}_{bi}")
                for j, t in enumerate(batch):
                    nc.tensor.matmul(
                        qk_ps[:, j * QC:(j + 1) * QC],
                        lhsT=mm1_lhsT(t),
                        rhs=st.q_rhs,
                        start=True,
                        stop=True,
                    )
                exp3 = workp.tile([128, 3 * QC], bf16, tag="exp3",
                                  name=f"exp{c}_{bi}")
                nc.scalar.activation(
                    exp3[:, :w], qk_ps[:, :w],
                    func=mybir.ActivationFunctionType.Exp,
                    bias=bias_m60,
                )
                if flush:
                    flush_mm2(st, flush_count)
                if bi in st.direct:
                    st.stashed[bi] = exp3
                else:
                    emit_chain(st, bi, exp3, w)
                if bi == st.nb - 1 and st.c == NQC - 1:
                    # the pre-final direct batch's row matmuls: its exp is
                    # done, so the row closes during this (last) exp
                    den_mm_exp(st, st.nb - 2)
                if bi == st.nb - 1 and st.den_emitted == st.den_total:
                    # den row complete: pull it to SBUF on the Scalar queue
                    # (idle after this exp) so it overlaps the DVE cast
                    epi_den_copy(st, on_scalar=True)
                st.mm2_pending.append((batch, exp3))
                if bi == st.nb - 1 and flush:
                    flush_mm2(st)
                    if c < NQC - 1:
                        epi_cast(st)

            # ---- epilogue stages ---------------------------------------
            def epi_fold0(st, part):
                # chunk-0 denominator on the DVE (its epilogue overlaps
                # chunk 1, where the DVE has slack and the PE does not):
                # fold the chain + direct tiles to a 512-wide acc_sum
                acc = st.accs[0]
                if part == 0:
                    st.acc_sum = epip.tile([128, QC], bf16, tag="acc_sum",
                                           name=f"accs{st.c}")
                    nc.vector.tensor_add(st.acc_sum, acc[:, 0:QC],
                                         acc[:, QC:2 * QC])
                    nc.vector.tensor_add(st.acc_sum, st.acc_sum,
                                         acc[:, 2 * QC:3 * QC])
                else:
                    e2 = st.stashed[st.nb - 2]
                    for sl in range(len(st.batches[st.nb - 2])):
                        nc.vector.tensor_add(st.acc_sum, st.acc_sum,
                                             e2[:, sl * QC:(sl + 1) * QC])
                    nc.vector.tensor_add(st.acc_sum, st.acc_sum,
                                         st.stashed[st.nb - 1][:, 0:QC])

            def epi_denom0(st):
                accT_ps = miscps.tile([128, QC], bf16, tag="misc",
                                      name=f"accT{st.c}")
                for s in range(4):
                    nc.tensor.transpose(
                        accT_ps[:, s * 128:(s + 1) * 128],
                        st.acc_sum[:, s * 128:(s + 1) * 128],
                        ident_bf,
                    )
                denom4 = epip.tile([128, 4], f32, tag="denom4",
                                   name=f"den4s{st.c}")
                nc.vector.tensor_reduce(
                    denom4,
                    accT_ps.rearrange("p (s j) -> p s j", s=4),
                    axis=mybir.AxisListType.X,
                    op=mybir.AluOpType.add,
                )
                st.recip4 = epip.tile([128, 4], f32, tag="recip4",
                                      name=f"rec{st.c}")
                nc.vector.reciprocal(st.recip4, denom4)

            def epi_cast(st, on_scalar=False):
                st.outT_sb = epip.tile([128, QC], bf16, tag="outT_sb",
                                       name=f"outTs{st.c}")
                if on_scalar:
                    nc.scalar.copy(st.outT_sb, st.outT_ps)
                else:
                    nc.vector.tensor_copy(st.outT_sb, st.outT_ps)

            def epi_den_copy(st, on_scalar=False):
                st.den_sb = epip.tile([1, QC], bf16, tag="den_sb",
                                      name=f"dens{st.c}")
                if on_scalar:
                    nc.scalar.copy(st.den_sb, st.den_ps)
                else:
                    nc.vector.tensor_copy(st.den_sb, st.den_ps)

            def epi_den4(st):
                # (128, 8) denominator halves: cols 0-3 from the den row via
                # K=1 extracts, cols 4-7 from the final batch's tile via
                # transposed ones-matmuls (every column is a self-contained
                # single-matmul group); one DVE reduce folds the halves,
                # then one reciprocal
                den4_ps = miscps.tile([128, 8], f32, tag="misc",
                                      name=f"den4{st.c}")
                e = st.stashed[st.nb - 1]
                for s in range(4):
                    nc.tensor.matmul(
                        den4_ps[:, 4 + s:5 + s],
                        lhsT=e[:, s * 128:(s + 1) * 128],
                        rhs=ones_col,
                        start=True,
                        stop=True,
                    )
                for s in range(4):
                    nc.tensor.matmul(
                        den4_ps[:, s:s + 1],
                        lhsT=st.den_sb[0:1, s * 128:(s + 1) * 128],
                        rhs=ones_1,
                        start=True,
                        stop=True,
                    )
                denom4 = epip.tile([128, 4], f32, tag="denom4",
                                   name=f"den4s{st.c}")
                nc.vector.tensor_reduce(
                    denom4,
                    den4_ps.rearrange("p (g s) -> p s g", g=2),
                    axis=mybir.AxisListType.X,
                    op=mybir.AluOpType.add,
                )
                st.recip4 = epip.tile([128, 4], f32, tag="recip4",
                                      name=f"rec{st.c}")
                nc.vector.reciprocal(st.recip4, denom4)

            def epi_outT_transpose(st, split=False):
                if split:
                    qa = miscps.tile([128, 256], bf16, tag="misc",
                                     name=f"outQa{st.c}")
                    qb = outps.tile([128, 256], bf16, tag="outT",
                                    name=f"outQb{st.c}")
                else:
                    qa = miscps.tile([128, QC], bf16, tag="misc",
                                     name=f"outQ{st.c}")
                    qb = None
                # (tile, col-block offset of q-block 2*half) per half
                st.outQ_parts = ((qa, 0), (qb, 0)) if split \
                    else ((qa, 0), (qa, 2))
                for s in range(4):
                    tgt, off = ((qa, s) if (not split or s < 2)
                                else (qb, s - 2))
                    nc.tensor.transpose(
                        tgt[:, off * 128:(off + 1) * 128],
                        st.outT_sb[:, s * 128:(s + 1) * 128],
                        ident_bf,
                    )
                # separate tiles per output half so the DVE and Scalar
                # scale pairs carry no false tile-level dependency
                st.out_sb = [
                    epip.tile([128, 2, 128], f32, tag=f"out_sb{h}",
                              name=f"outs{st.c}_{h}")
                    for h in range(2)
                ]

            def epi_scale_dma(st, half, on_scalar=False):
                c = st.c
                ob = st.out_sb[half]
                qt, base = st.outQ_parts[half]
                for j, s in enumerate((2 * half, 2 * half + 1)):
                    qs = qt[:, (base + j) * 128:(base + j + 1) * 128]
                    if on_scalar:
                        nc.scalar.mul(ob[:, j, :], qs,
                                      st.recip4[:, s:s + 1])
                    else:
                        nc.vector.tensor_scalar_mul(ob[:, j, :], qs,
                                                    st.recip4[:, s:s + 1])
                nc.sync.dma_start(
                    out=out_ext[c * QC + half * 256:c * QC + (half + 1) * 256,
                                :].rearrange("(s i) j -> i s j", s=2),
                    in_=ob,
                )

            # ---- software-pipelined chunk schedule --------------------
            st = start_chunk(0)
            emit_batch(st, 0)
            emit_batch(st, 1, flush=False)
            emit_batch(st, 2, flush=False)
            # lag-2 mm2 flushing while the vt stream catches up; catch-up
            # flushes mid-chunk shrink the boundary backlog to one batch
            for bi in range(3, st.nb - 1):
                emit_batch(st, bi,
                           flush_count=2 if bi in (12, 16, 20, 21) else 1)
            emit_batch(st, st.nb - 1, flush=False)
            st1 = start_chunk(1)
            emit_batch(st1, 0, flush=False)
            emit_batch(st1, 1, flush=False)
            flush_mm2(st)          # chunk 0's mm2 backlog
            epi_cast(st)
            emit_batch(st1, 2, flush=False)
            epi_fold0(st, 0)       # chunk-0 denominator on the DVE,
            emit_batch(st1, 3, flush=False)  # spread over c1 batches
            epi_fold0(st, 1)
            flush_mm2(st1, 2)      # mm2(c1 b0), mm2(c1 b1)
            emit_batch(st1, 4, flush_count=2)
            emit_batch(st1, 5)     # cadence restored
            epi_denom0(st)
            epi_outT_transpose(st)
            emit_batch(st1, 6)
            emit_batch(st1, 7)
            epi_scale_dma(st, 0)
            emit_batch(st1, 8)
            epi_scale_dma(st, 1)
            for bi in range(9, st1.nb):
                emit_batch(st1, bi)
            # final tail (den-copy holds the Scalar queue, cast on DVE)
            epi_cast(st1)
            epi_den4(st1)
            epi_outT_transpose(st1, split=True)
            epi_scale_dma(st1, 0)
            epi_scale_dma(st1, 1, on_scalar=True)
    return nc


def kernel(q, k, v):
    global LAST_RESULTS
    from concourse.bass_utils import run_bass_kernel_spmd

    q = np.ascontiguousarray(np.asarray(q, dtype=np.float32))
    k = np.ascontiguousarray(np.asarray(k, dtype=np.float32))
    v = np.ascontiguousarray(np.asarray(v, dtype=np.float32))

    # host-side layout prep: vt[p, 128t+c] = v[c, 128t+p] -- the exact SBUF
    # layout mm2 wants for its stationary operand (zero device transposes).
    vt = np.ascontiguousarray(
        v.reshape(D, NKV, KVT).transpose(2, 1, 0).reshape(D, SKV)
    )

    nc = _build_nc()
    nc.finalize()
    in_maps = [
        {
            "q": np.ascontiguousarray(q[:, i * SQS:(i + 1) * SQS]),
            "k": k,
            "vt": vt,
        }
        for i in range(NCORES)
    ]
    res = run_bass_kernel_spmd(nc, in_maps, core_ids=list(range(NCORES)))
    LAST_RESULTS = res
    out = np.concatenate([res.results[i]["out"] for i in range(NCORES)], axis=0)
    return out.astype(np.float32)
